# revision 35
# baseline (speedup 1.0000x reference)
"""CondConv2d (MoE routed conv) Trainium2 kernel.

Math: out[b] = sum_e routing[b,e] * conv3x3(x[b], W[e])
Since the expert mix is linear in W, this equals
    out[b] = conv3x3(x[b], Wmix_b),  Wmix_b = sum_e routing[b,e] * W[e]
which needs 1 conv per sample instead of E=4 (4x less PE work).

Sharding: data-parallel over batch, B=16 -> 2 samples per core on 8 cores.
Weights (all 4 experts, transposed to [ci, tap, e, co] on host) are
replicated; the per-sample mix happens on-device on the Vector engine.

Conv as implicit GEMM: x is zero-padded on host to [ci, 58, 58]; for each
of 9 taps the matmul streams a shifted window of the padded image
(rhs = xpad[:, blk*8+kh : +8, kw : kw+56], N=448) against the tap's mixed
weight slice (lhsT = Wmix[ci, co], K=ci on partitions), accumulating all
9 taps into one PSUM bank. 7 row-blocks of 8 rows cover the 56 output
rows. Matmuls run as float32r (1 cycle/row at N>=256 vs 4 for fp32);
fp32r is fp32 with the mantissa rounded to 11 bits, ~16x more accurate
than bf16. x is pre-rounded to fp32r on the host; the weight mix is
rounded by the DVE output cast.

Schedule: sample 0 runs tap-outer (all 7 PSUM banks accumulate one tap at
a time) so matmuls start after only the first tap's weights + first x
rows arrive. Weights go on the scalar-engine DMA ring, x chunks on the
sync ring, so the first tap's weights and first x rows land in parallel.
Sample 1 runs block-outer (9 taps into one bank, then drain) so the
output streams out incrementally, with a small final block to shorten the
tail; its weight mix runs on GpSimd to keep DVE free for sample 0's tap
mixes and the PSUM drains. Dummy matmuls on a zeroed tile during the load
phase keep the PE HAM clock-gate warm (2.4 GHz) for the real stream.
"""

import os
import sys

os.environ.setdefault("MYCRO_LOCAL_CACHE", "1")
for _p in ("/opt/trn_rl_repo",):
    if _p not in sys.path:
        sys.path.insert(0, _p)

import numpy as np

B, CIN, COUT, H, W_SP = 16, 128, 128, 56, 56
E, KH, KW = 4, 3, 3
NCORES = 8
SPC = B // NCORES          # samples per core
HP, WP = H + 2, W_SP + 2   # padded spatial
NTAP = KH * KW
RPB = 8                    # output rows per matmul block
NBLK = H // RPB
NT = RPB * W_SP            # moving-operand free size per matmul (448)
N_WARM = 12                # HAM warm-up dummy matmuls
# sample-0 mix chunks (start_tap, n_taps): per-tap for the first taps so
# the tap-outer stream can start ASAP; each chunk gets its OWN tile
# (matmul weight reads are tracked whole-tile, so chunks sharing a tile
# serialize behind all earlier matmuls)
MIXCH = [(0, 1), (1, 1), (2, 1), (3, 3), (6, 3)]

# x chunks (padded-row ranges); a block of rows [r0, r0+nr) needs padded
# rows [r0, r0+nr+2). Sample 0 uses fine chunks so the first matmuls gate
# on as few bytes as possible; sample 1 loads well ahead, coarser is fine.
XCH0 = [(0, 18), (16, 12), (24, 18), (40, 18)]
BLK_CH0 = [0, 0, 1, 2, 2, 3, 3]            # 8-row block -> chunk
XCH1 = [(0, 26), (24, 18), (40, 18)]
BLK_CH1 = [0, 0, 0, 1, 1, 2, 2]
# sample-1 row blocks (start_row, n_rows); all >=256 moving cols (f32r
# matmuls at N<256 pay 4 cycles/row)
BLKS1 = [(8 * b, 8) for b in range(NBLK)]

_cached_nc = None


def _round_f32r(a):
    """Round fp32 array to fp32r bits (RNE to 11 mantissa bits)."""
    u = a.view(np.uint32)
    lsb = (u >> np.uint32(12)) & np.uint32(1)
    return ((u + np.uint32(0x7FF) + lsb) & np.uint32(0xFFFFF000)).view(np.float32)


def _build_nc():
    import concourse.tile as tile
    from concourse import bacc, mybir

    f32 = mybir.dt.float32
    f32r = mybir.dt.float32r
    MUL, ADD = mybir.AluOpType.mult, mybir.AluOpType.add

    nc = bacc.Bacc(
        "TRN2", target_bir_lowering=False, debug=False, num_devices=NCORES
    )

    xpad_d = nc.dram_tensor(
        "xpad", [SPC, CIN, HP * WP], f32r, kind="ExternalInput"
    ).ap()
    # host layout: [ci, (rb | tap, e, co)] — routing scalars share the
    # weight tensor so one DMA delivers both rb and the first tap
    TAPW = E * COUT  # 512 floats per tap in wt
    RBW = SPC * E
    wt_d = nc.dram_tensor(
        "wt", [CIN, RBW + NTAP * TAPW], f32, kind="ExternalInput"
    ).ap()
    out_d = nc.dram_tensor(
        "out", [SPC, COUT, H * W_SP], f32, kind="ExternalOutput"
    ).ap()

    with tile.TileContext(nc) as tc:
        with (
            tc.tile_pool(name="const", bufs=1) as cst,
            tc.tile_pool(name="x", bufs=2) as xpool,
            tc.tile_pool(name="wmix", bufs=2) as wmp,
            tc.tile_pool(name="ob", bufs=3) as opool,
            tc.tile_pool(name="ps", bufs=8, space="PSUM") as pspool,
        ):
            # --- HAM warm-up: dummy matmuls on a zeroed tile during loads
            # (bf16: memset doesn't support f32r, and bf16 streams 1 cyc/row)
            zt = cst.tile([128, 512], mybir.dt.bfloat16, tag="zero")
            nc.gpsimd.memset(zt[:], 0.0)
            warm_ps = pspool.tile([128, 512], f32, tag="ps")
            for _ in range(N_WARM):
                nc.tensor.matmul(
                    warm_ps[:], zt[:, :128], zt[:], start=True, stop=True
                )

            # weights + routing on the scalar-engine DMA ring; x on the
            # sync ring — the critical first pieces land in parallel
            wt_t = cst.tile([CIN, RBW + NTAP * TAPW], f32, tag="wt")
            rb_t = wt_t[:, 0:RBW]

            def load_wt_tap(t, eng):
                # first chunk also carries the routing scalars
                lo = 0 if t == 0 else RBW + t * TAPW
                sl = slice(lo, RBW + (t + 1) * TAPW)
                eng.dma_start(wt_t[:, sl], wt_d[:, sl])

            def load_x_chunk(s, xtiles, xch, c, eng):
                r0, nr = xch[c]
                xt = xpool.tile([CIN, nr * WP], f32r, tag=f"x{s}_{c}",
                                name=f"x{s}_{c}")
                sl = slice(r0 * WP, (r0 + nr) * WP)
                eng.dma_start(xt[:], xpad_d[s][:, sl])
                xtiles[c] = xt

            # scalar ring: the critical first weights, then sample 1's x.
            # sync ring: sample 0's x chunks, then the later weight taps.
            # The rings share HBM bandwidth, so each ring's early entries
            # are exactly what gates the next phase of the PE stream.
            x0t = [None] * len(XCH0)
            x1t = [None] * len(XCH1)
            for t in range(3):
                load_wt_tap(t, nc.scalar)
            for c in range(len(XCH0)):
                load_x_chunk(0, x0t, XCH0, c, nc.sync)
            for c in range(len(XCH1)):
                load_x_chunk(1, x1t, XCH1, c, nc.scalar)
            for t in range(3, NTAP):
                load_wt_tap(t, nc.sync)

            wt3 = wt_t[:, RBW:].rearrange("p (t e c) -> p t e c", t=NTAP, e=E)

            def mix(dst3, s, t0, t1, eng):
                """dst3[:, :, :] = sum_e rb[s,e] * wt[:, t0:t1, e, :]"""
                for e in range(E):
                    sc = rb_t[:, s * E + e : s * E + e + 1]
                    src = wt3[:, t0:t1, e, :]
                    if e == 0:
                        eng.tensor_scalar_mul(dst3, src, sc)
                    else:
                        eng.scalar_tensor_tensor(dst3, src, sc, dst3, MUL, ADD)

            def rhs_ap(xtiles, c, r0, nr, kh, kw):
                xch = XCH0 if xtiles is x0t else XCH1
                loc = r0 - xch[c][0]
                x3 = xtiles[c][:].rearrange("p (h w) -> p h w", w=WP)
                return x3[:, loc + kh : loc + kh + nr, kw : kw + W_SP]

            def store_block(s, ob, ps, r0, nr):
                sl = slice(r0 * W_SP, (r0 + nr) * W_SP)
                nc.vector.tensor_copy(ob[:, sl], ps[:])
                nc.sync.dma_start(out_d[s][:, sl], ob[:, sl])

            # ---- sample 0: tap-outer over 7 live PSUM banks
            wm0 = {}  # tap -> (chunk AP, local tap index)
            ps_map = {}
            for blk in range(NBLK):
                ps_map[blk] = pspool.tile(
                    [COUT, NT], f32, tag="ps", name=f"ps0_{blk}"
                )
            def mix_chunk(c):
                t0, ntc = MIXCH[c]
                wmt = wmp.tile(
                    [CIN, ntc * COUT], f32r, tag=f"wmc{c}", name=f"wm0_{c}"
                )
                wm3 = wmt.rearrange("p (t c) -> p t c", t=ntc)
                mix(wm3, 0, t0, t0 + ntc, nc.vector)
                for tt in range(t0, t0 + ntc):
                    wm0[tt] = (wmt, tt - t0)

            def mm0(t, blk):
                kh, kw = divmod(t, KW)
                chunk, loc = wm0[t]
                nc.tensor.matmul(
                    ps_map[blk][:],
                    chunk[:, loc * COUT : (loc + 1) * COUT],
                    rhs_ap(x0t, BLK_CH0[blk], blk * RPB, RPB, kh, kw),
                    start=(t == 0),
                    stop=(t == NTAP - 1),
                    skip_group_check=True,
                )

            # phase 1: taps 0-2 on the first x chunk's blocks — starts as
            # soon as the first tap's weights + first 18 x rows land
            for t in range(3):
                mix_chunk(t)
                for blk in range(2):
                    mm0(t, blk)
            # phase 2: taps 0-2 on the later blocks, block-major to track
            # the arrival of the remaining x chunks
            for blk in range(2, NBLK):
                for t in range(3):
                    mm0(t, blk)
            # phase 3: taps 3-8 everywhere; both remaining mix chunks are
            # emitted up front so DVE finishes them well before they're
            # needed (and before sample 1's mix)
            mix_chunk(3)
            mix_chunk(4)
            for t in range(3, NTAP):
                for blk in range(NBLK):
                    mm0(t, blk)

            # sample 1 weight mix: runs on DVE during sample 0's stream
            wm1 = wmp.tile([CIN, NTAP * COUT], f32r, tag="wm")
            wm1_3 = wm1[:].rearrange("p (t c) -> p t c", t=NTAP)
            mix(wm1_3, 1, 0, NTAP, nc.vector)

            # drain sample 0
            ob0 = opool.tile([COUT, H * W_SP], f32, tag="ob")
            for blk in range(NBLK):
                store_block(0, ob0, ps_map[blk], blk * RPB, RPB)

            # ---- sample 1: block-outer, drains incrementally
            ob1 = opool.tile([COUT, H * W_SP], f32, tag="ob")
            for blk, (r0, nr) in enumerate(BLKS1):
                ps = pspool.tile(
                    [COUT, nr * W_SP], f32, tag="ps", name=f"ps1_{blk}"
                )
                for t in range(NTAP):
                    kh, kw = divmod(t, KW)
                    nc.tensor.matmul(
                        ps[:],
                        wm1[:, t * COUT : (t + 1) * COUT],
                        rhs_ap(x1t, BLK_CH1[blk], r0, nr, kh, kw),
                        start=(t == 0),
                        stop=(t == NTAP - 1),
                    )
                store_block(1, ob1, ps, r0, nr)

    nc.compile()
    return nc


def _get_nc():
    global _cached_nc
    if _cached_nc is None:
        _cached_nc = _build_nc()
    return _cached_nc


def _prep_inputs(x, routing_weights, W):
    x = np.ascontiguousarray(x, dtype=np.float32)
    routing_weights = np.ascontiguousarray(routing_weights, dtype=np.float32)
    W = np.ascontiguousarray(W, dtype=np.float32)

    xpad = np.zeros((B, CIN, HP, WP), np.float32)
    xpad[:, :, 1 : H + 1, 1 : W_SP + 1] = _round_f32r(x.reshape(B, CIN, H, W_SP))
    xpad = xpad.reshape(B, CIN, HP * WP)

    # W[e, co, ci, kh, kw] -> wt[ci, (kh, kw, e, co)], with the per-core
    # routing scalars (broadcast over partitions) prepended
    wt = np.ascontiguousarray(np.transpose(W, (2, 3, 4, 0, 1))).reshape(
        CIN, NTAP * E * COUT
    )

    in_maps = []
    for c in range(NCORES):
        r = routing_weights[c * SPC : (c + 1) * SPC]  # [SPC, E]
        rb = np.broadcast_to(r.reshape(1, SPC * E), (128, SPC * E))
        in_maps.append(
            {
                "xpad": xpad[c * SPC : (c + 1) * SPC],
                "wt": np.ascontiguousarray(np.concatenate([rb, wt], axis=1)),
            }
        )
    return in_maps


def _run(in_maps, **kwargs):
    from concourse import bass_utils

    nc = _get_nc()
    res = bass_utils.run_bass_kernel_spmd(
        nc, in_maps, core_ids=list(range(NCORES)), **kwargs
    )
    out = np.concatenate(
        [res.results[c]["out"] for c in range(NCORES)], axis=0
    ).reshape(B, COUT, H, W_SP)
    return out, res


def kernel(x, routing_weights, W):
    in_maps = _prep_inputs(x, routing_weights, W)
    out, _ = _run(in_maps)
    return out


# revision 39
# speedup vs baseline: 1.0407x; 1.0407x over previous
"""CondConv2d (MoE routed conv) Trainium2 kernel.

Math: out[b] = sum_e routing[b,e] * conv3x3(x[b], W[e])
Since the expert mix is linear in W, this equals
    out[b] = conv3x3(x[b], Wmix_b),  Wmix_b = sum_e routing[b,e] * W[e]
which needs 1 conv per sample instead of E=4 (4x less PE work).

Sharding: data-parallel over batch, B=16 -> 2 samples per core on 8 cores.
Weights (all 4 experts, transposed to [ci, tap, e, co] on host) are
replicated; the per-sample mix happens on-device on the Vector engine.

Conv as implicit GEMM: x is zero-padded on host to [ci, 58, 58]; for each
of 9 taps the matmul streams a shifted window of the padded image
(rhs = xpad[:, blk*8+kh : +8, kw : kw+56], N=448) against the tap's mixed
weight slice (lhsT = Wmix[ci, co], K=ci on partitions), accumulating all
9 taps into one PSUM bank. 7 row-blocks of 8 rows cover the 56 output
rows. Matmuls run as float32r (1 cycle/row at N>=256 vs 4 for fp32);
fp32r is fp32 with the mantissa rounded to 11 bits, ~16x more accurate
than bf16. x is pre-rounded to fp32r on the host; the weight mix is
rounded by the DVE output cast.

Schedule: sample 0 runs tap-outer (all 7 PSUM banks accumulate one tap at
a time) so matmuls start after only the first tap's weights + first x
rows arrive. Weights go on the scalar-engine DMA ring, x chunks on the
sync ring, so the first tap's weights and first x rows land in parallel.
Sample 1 runs block-outer (9 taps into one bank, then drain) so the
output streams out incrementally, with a small final block to shorten the
tail; its weight mix runs on GpSimd to keep DVE free for sample 0's tap
mixes and the PSUM drains. Dummy matmuls on a zeroed tile during the load
phase keep the PE HAM clock-gate warm (2.4 GHz) for the real stream.
"""

import os
import sys

os.environ.setdefault("MYCRO_LOCAL_CACHE", "1")
for _p in ("/opt/trn_rl_repo",):
    if _p not in sys.path:
        sys.path.insert(0, _p)

import numpy as np

B, CIN, COUT, H, W_SP = 16, 128, 128, 56, 56
E, KH, KW = 4, 3, 3
NCORES = 8
SPC = B // NCORES          # samples per core
HP, WP = H + 2, W_SP + 2   # padded spatial
NTAP = KH * KW
RPB = 8                    # output rows per matmul block
NBLK = H // RPB
NT = RPB * W_SP            # moving-operand free size per matmul (448)
N_WARM = 11                # HAM warm-up dummy matmuls
# sample-0 mix chunks (start_tap, n_taps): per-tap for the first taps so
# the tap-outer stream can start ASAP; each chunk gets its OWN tile
# (matmul weight reads are tracked whole-tile, so chunks sharing a tile
# serialize behind all earlier matmuls)
MIXCH = [(0, 1), (1, 1), (2, 1), (3, 1), (4, 2), (6, 3)]

# x chunks (padded-row ranges); a block of rows [r0, r0+nr) needs padded
# rows [r0, r0+nr+2). Sample 0 uses fine chunks so the first matmuls gate
# on as few bytes as possible; sample 1 loads well ahead, coarser is fine.
XCH0 = [(0, 18), (16, 12), (24, 18), (40, 18)]
BLK_CH0 = [0, 0, 1, 2, 2, 3, 3]            # 8-row block -> chunk
XCH1 = [(0, 26), (24, 18), (40, 18)]
BLK_CH1 = [0, 0, 0, 1, 1, 2, 2]
# sample-1 row blocks (start_row, n_rows); all >=256 moving cols (f32r
# matmuls at N<256 pay 4 cycles/row)
BLKS1 = [(8 * b, 8) for b in range(NBLK)]

_cached_nc = None


def _round_f32r(a):
    """Round fp32 array to fp32r bits (RNE to 11 mantissa bits)."""
    u = a.view(np.uint32)
    lsb = (u >> np.uint32(12)) & np.uint32(1)
    return ((u + np.uint32(0x7FF) + lsb) & np.uint32(0xFFFFF000)).view(np.float32)


def _build_nc():
    import concourse.tile as tile
    from concourse import bacc, mybir

    f32 = mybir.dt.float32
    f32r = mybir.dt.float32r
    MUL, ADD = mybir.AluOpType.mult, mybir.AluOpType.add

    nc = bacc.Bacc(
        "TRN2", target_bir_lowering=False, debug=False, num_devices=NCORES
    )

    xpad_d = nc.dram_tensor(
        "xpad", [SPC, CIN, HP * WP], f32r, kind="ExternalInput"
    ).ap()
    # host layout: [ci, (rb | tap, e, co)] — routing scalars share the
    # weight tensor so one DMA delivers both rb and the first tap
    TAPW = E * COUT  # 512 floats per tap in wt
    RBW = SPC * E
    wt_d = nc.dram_tensor(
        "wt", [CIN, RBW + NTAP * TAPW], f32, kind="ExternalInput"
    ).ap()
    out_d = nc.dram_tensor(
        "out", [SPC, COUT, H * W_SP], f32, kind="ExternalOutput"
    ).ap()

    with tile.TileContext(nc) as tc:
        with (
            tc.tile_pool(name="const", bufs=1) as cst,
            tc.tile_pool(name="x", bufs=2) as xpool,
            tc.tile_pool(name="wmix", bufs=2) as wmp,
            tc.tile_pool(name="ob", bufs=3) as opool,
            tc.tile_pool(name="ps", bufs=8, space="PSUM") as pspool,
        ):
            # --- HAM warm-up: dummy matmuls on a zeroed tile during loads
            # (bf16: memset doesn't support f32r, and bf16 streams 1 cyc/row)
            zt = cst.tile([128, 512], mybir.dt.bfloat16, tag="zero")
            nc.gpsimd.memset(zt[:], 0.0)
            warm_ps = pspool.tile([128, 512], f32, tag="ps")
            for _ in range(N_WARM):
                nc.tensor.matmul(
                    warm_ps[:], zt[:, :128], zt[:], start=True, stop=True
                )

            # weights + routing on the scalar-engine DMA ring; x on the
            # sync ring — the critical first pieces land in parallel
            wt_t = cst.tile([CIN, RBW + NTAP * TAPW], f32, tag="wt")
            rb_t = wt_t[:, 0:RBW]

            def load_wt_tap(t, eng):
                # first chunk also carries the routing scalars
                lo = 0 if t == 0 else RBW + t * TAPW
                sl = slice(lo, RBW + (t + 1) * TAPW)
                eng.dma_start(wt_t[:, sl], wt_d[:, sl])

            def load_x_chunk(s, xtiles, xch, c, eng):
                r0, nr = xch[c]
                xt = xpool.tile([CIN, nr * WP], f32r, tag=f"x{s}_{c}",
                                name=f"x{s}_{c}")
                sl = slice(r0 * WP, (r0 + nr) * WP)
                eng.dma_start(xt[:], xpad_d[s][:, sl])
                xtiles[c] = xt

            # scalar ring: the critical first weights, then sample 1's x.
            # sync ring: sample 0's x chunks, then the later weight taps.
            # The rings share HBM bandwidth, so each ring's early entries
            # are exactly what gates the next phase of the PE stream.
            # Both rings round-robin at packet granularity and share HBM
            # bandwidth, so each ring's FIFO must be ordered by global
            # need-time; urgent pieces are split across the two rings.
            x0t = [None] * len(XCH0)
            x1t = [None] * len(XCH1)
            load_wt_tap(0, nc.scalar)
            load_x_chunk(0, x0t, XCH0, 0, nc.sync)
            load_wt_tap(1, nc.scalar)
            load_x_chunk(0, x0t, XCH0, 1, nc.sync)
            load_wt_tap(2, nc.scalar)
            load_x_chunk(0, x0t, XCH0, 3, nc.sync)
            load_x_chunk(0, x0t, XCH0, 2, nc.scalar)
            load_wt_tap(5, nc.sync)
            load_wt_tap(3, nc.scalar)
            load_wt_tap(4, nc.scalar)
            load_wt_tap(7, nc.sync)
            load_wt_tap(6, nc.scalar)
            load_wt_tap(8, nc.scalar)
            load_x_chunk(1, x1t, XCH1, 0, nc.scalar)
            load_x_chunk(1, x1t, XCH1, 1, nc.sync)
            load_x_chunk(1, x1t, XCH1, 2, nc.scalar)

            wt3 = wt_t[:, RBW:].rearrange("p (t e c) -> p t e c", t=NTAP, e=E)

            def mix(dst3, s, t0, t1, eng):
                """dst3[:, :, :] = sum_e rb[s,e] * wt[:, t0:t1, e, :]"""
                for e in range(E):
                    sc = rb_t[:, s * E + e : s * E + e + 1]
                    src = wt3[:, t0:t1, e, :]
                    if e == 0:
                        eng.tensor_scalar_mul(dst3, src, sc)
                    else:
                        eng.scalar_tensor_tensor(dst3, src, sc, dst3, MUL, ADD)

            def rhs_ap(xtiles, c, r0, nr, kh, kw):
                xch = XCH0 if xtiles is x0t else XCH1
                loc = r0 - xch[c][0]
                x3 = xtiles[c][:].rearrange("p (h w) -> p h w", w=WP)
                return x3[:, loc + kh : loc + kh + nr, kw : kw + W_SP]

            def store_block(s, ob, ps, r0, nr):
                sl = slice(r0 * W_SP, (r0 + nr) * W_SP)
                nc.vector.tensor_copy(ob[:, sl], ps[:])
                nc.sync.dma_start(out_d[s][:, sl], ob[:, sl])

            # ---- sample 0: tap-outer over 7 live PSUM banks
            wm0 = {}  # tap -> (chunk AP, local tap index)
            ps_map = {}
            for blk in range(NBLK):
                ps_map[blk] = pspool.tile(
                    [COUT, NT], f32, tag="ps", name=f"ps0_{blk}"
                )
            def mix_chunk(c):
                t0, ntc = MIXCH[c]
                wmt = wmp.tile(
                    [CIN, ntc * COUT], f32r, tag=f"wmc{c}", name=f"wm0_{c}"
                )
                wm3 = wmt.rearrange("p (t c) -> p t c", t=ntc)
                mix(wm3, 0, t0, t0 + ntc, nc.vector)
                for tt in range(t0, t0 + ntc):
                    wm0[tt] = (wmt, tt - t0)

            def mm0(t, blk):
                kh, kw = divmod(t, KW)
                chunk, loc = wm0[t]
                nc.tensor.matmul(
                    ps_map[blk][:],
                    chunk[:, loc * COUT : (loc + 1) * COUT],
                    rhs_ap(x0t, BLK_CH0[blk], blk * RPB, RPB, kh, kw),
                    start=(t == 0),
                    stop=(t == NTAP - 1),
                    skip_group_check=True,
                )

            # phase 1: taps 0-2 on the first x chunk's blocks — starts as
            # soon as the first tap's weights + first 18 x rows land
            for t in range(3):
                mix_chunk(t)
                for blk in range(2):
                    mm0(t, blk)
            # phase 2: taps 0-2 on the later blocks, block-major to track
            # the arrival of the remaining x chunks
            for blk in range(2, NBLK):
                for t in range(3):
                    mm0(t, blk)
            # phase 3: taps 3-8 everywhere; remaining mix chunks are
            # emitted up front so DVE finishes them well before they're
            # needed (and before sample 1's mix)
            for c in range(3, len(MIXCH)):
                mix_chunk(c)
            for t in range(3, NTAP):
                for blk in range(NBLK):
                    mm0(t, blk)

            # sample 1 weight mix: runs on DVE during sample 0's stream
            wm1 = wmp.tile([CIN, NTAP * COUT], f32r, tag="wm")
            wm1_3 = wm1[:].rearrange("p (t c) -> p t c", t=NTAP)
            mix(wm1_3, 1, 0, NTAP, nc.vector)

            # drain sample 0
            ob0 = opool.tile([COUT, H * W_SP], f32, tag="ob")
            for blk in range(NBLK):
                store_block(0, ob0, ps_map[blk], blk * RPB, RPB)

            # ---- sample 1: block-outer, drains incrementally
            ob1 = opool.tile([COUT, H * W_SP], f32, tag="ob")
            for blk, (r0, nr) in enumerate(BLKS1):
                ps = pspool.tile(
                    [COUT, nr * W_SP], f32, tag="ps", name=f"ps1_{blk}"
                )
                for t in range(NTAP):
                    kh, kw = divmod(t, KW)
                    nc.tensor.matmul(
                        ps[:],
                        wm1[:, t * COUT : (t + 1) * COUT],
                        rhs_ap(x1t, BLK_CH1[blk], r0, nr, kh, kw),
                        start=(t == 0),
                        stop=(t == NTAP - 1),
                    )
                store_block(1, ob1, ps, r0, nr)

    nc.compile()
    return nc


def _get_nc():
    global _cached_nc
    if _cached_nc is None:
        _cached_nc = _build_nc()
    return _cached_nc


def _prep_inputs(x, routing_weights, W):
    x = np.ascontiguousarray(x, dtype=np.float32)
    routing_weights = np.ascontiguousarray(routing_weights, dtype=np.float32)
    W = np.ascontiguousarray(W, dtype=np.float32)

    xpad = np.zeros((B, CIN, HP, WP), np.float32)
    xpad[:, :, 1 : H + 1, 1 : W_SP + 1] = _round_f32r(x.reshape(B, CIN, H, W_SP))
    xpad = xpad.reshape(B, CIN, HP * WP)

    # W[e, co, ci, kh, kw] -> wt[ci, (kh, kw, e, co)], with the per-core
    # routing scalars (broadcast over partitions) prepended
    wt = np.ascontiguousarray(np.transpose(W, (2, 3, 4, 0, 1))).reshape(
        CIN, NTAP * E * COUT
    )

    in_maps = []
    for c in range(NCORES):
        r = routing_weights[c * SPC : (c + 1) * SPC]  # [SPC, E]
        rb = np.broadcast_to(r.reshape(1, SPC * E), (128, SPC * E))
        in_maps.append(
            {
                "xpad": xpad[c * SPC : (c + 1) * SPC],
                "wt": np.ascontiguousarray(np.concatenate([rb, wt], axis=1)),
            }
        )
    return in_maps


def _run(in_maps, **kwargs):
    from concourse import bass_utils

    nc = _get_nc()
    res = bass_utils.run_bass_kernel_spmd(
        nc, in_maps, core_ids=list(range(NCORES)), **kwargs
    )
    out = np.concatenate(
        [res.results[c]["out"] for c in range(NCORES)], axis=0
    ).reshape(B, COUT, H, W_SP)
    return out, res


def kernel(x, routing_weights, W):
    in_maps = _prep_inputs(x, routing_weights, W)
    out, _ = _run(in_maps)
    return out


# revision 43
# speedup vs baseline: 1.2127x; 1.1653x over previous
"""CondConv2d (MoE routed conv) Trainium2 kernel.

Math: out[b] = sum_e routing[b,e] * conv3x3(x[b], W[e])
Since the expert mix is linear in W, this equals
    out[b] = conv3x3(x[b], Wmix_b),  Wmix_b = sum_e routing[b,e] * W[e]
which needs 1 conv per sample instead of E=4 (4x less PE work).

Sharding: data-parallel over batch, B=16 -> 2 samples per core on 8 cores.
Weights (all 4 experts, transposed to [ci, tap, e, co] on host) are
replicated; the per-sample mix happens on-device on the Vector engine.

Conv as implicit GEMM: x is zero-padded on host to [ci, 58, 58]; for each
of 9 taps the matmul streams a shifted window of the padded image
(rhs = xpad[:, blk*8+kh : +8, kw : kw+56], N=448) against the tap's mixed
weight slice (lhsT = Wmix[ci, co], K=ci on partitions), accumulating all
9 taps into one PSUM bank (fp32). 7 row-blocks of 8 rows cover the 56
output rows.

Numerics: x and W are rounded to fp16 on the host; matmuls run fp16 at
1 cycle/row with fp32 PSUM accumulation. fp16's 10-bit mantissa gives
~4e-4 L2 relative error on this problem, and halves the load bytes, the
DVE mix cost (16-bit 2x mode), and the weight-load time (FWL) compared
to the fp32 path.

Schedule: sample 0 runs tap-outer (all 7 PSUM banks accumulate one tap at
a time) so matmuls start after only the first tap's weights + first x
rows arrive; loads are chunked and spread across the sync and scalar DMA
rings in global need-time order. Sample 1 runs block-outer (9 taps into
one bank, then drain) so its output streams out incrementally; its weight
mix runs on DVE during sample 0's stream. Dummy matmuls on a zeroed tile
during the load phase keep the PE HAM clock-gate warm (2.4 GHz).
"""

import os
import sys

os.environ.setdefault("MYCRO_LOCAL_CACHE", "1")
for _p in ("/opt/trn_rl_repo",):
    if _p not in sys.path:
        sys.path.insert(0, _p)

import numpy as np

B, CIN, COUT, H, W_SP = 16, 128, 128, 56, 56
E, KH, KW = 4, 3, 3
NCORES = 8
SPC = B // NCORES          # samples per core
HP, WP = H + 2, W_SP + 2   # padded spatial
NTAP = KH * KW
RPB = 8                    # output rows per matmul block
NBLK = H // RPB
NT = RPB * W_SP            # moving-operand free size per matmul (448)
N_WARM = 9                 # HAM warm-up dummy matmuls

# sample-0 mix chunks (start_tap, n_taps): per-tap early for latency, then
# wider; each chunk gets its OWN tile (matmul weight reads are tracked
# whole-tile, so chunks sharing a tile serialize behind earlier matmuls)
MIXCH = [(0, 1), (1, 1), (2, 1), (3, 3), (6, 3)]
# x chunks (padded-row ranges); a block of rows [r0, r0+nr) needs padded
# rows [r0, r0+nr+2)
XCH0 = [(0, 18), (16, 12), (24, 18), (40, 18)]
BLK_CH0 = [0, 0, 1, 2, 2, 3, 3]            # 8-row block -> chunk
XCH1 = [(0, 26), (24, 18), (40, 18)]
BLK_CH1 = [0, 0, 0, 1, 1, 2, 2]
BLKS1 = [(8 * b, 8) for b in range(NBLK)]

_cached_nc = None


def _build_nc():
    import concourse.tile as tile
    from concourse import bacc, mybir

    f32 = mybir.dt.float32
    f16 = mybir.dt.float16
    MUL, ADD = mybir.AluOpType.mult, mybir.AluOpType.add

    nc = bacc.Bacc(
        "TRN2", target_bir_lowering=False, debug=False, num_devices=NCORES
    )

    xpad_d = nc.dram_tensor(
        "xpad", [SPC, CIN, HP * WP], f16, kind="ExternalInput"
    ).ap()
    # host layout: [ci, (rb | tap, e, co)] — routing scalars (fp32 bits
    # packed into 2 fp16 slots each; tensor_scalar wants fp32 scalars)
    # share the weight tensor so one DMA delivers both rb and tap 0
    TAPW = E * COUT          # 512 halfs per tap in wt
    RBW = SPC * E * 2        # fp32 scalars as fp16 slot pairs
    wt_d = nc.dram_tensor(
        "wt", [CIN, RBW + NTAP * TAPW], f16, kind="ExternalInput"
    ).ap()
    out_d = nc.dram_tensor(
        "out", [SPC, COUT, H * W_SP], f32, kind="ExternalOutput"
    ).ap()

    with tile.TileContext(nc) as tc:
        with (
            tc.tile_pool(name="const", bufs=1) as cst,
            tc.tile_pool(name="x", bufs=1) as xpool,
            tc.tile_pool(name="wmix", bufs=1) as wmp,
            tc.tile_pool(name="ob", bufs=2) as opool,
            tc.tile_pool(name="ps", bufs=8, space="PSUM") as pspool,
        ):
            # --- HAM warm-up: dummy matmuls on a zeroed tile during loads
            zt = cst.tile([128, 512], f16, tag="zero")
            nc.gpsimd.memset(zt[:], 0.0)
            warm_ps = pspool.tile([128, 512], f32, tag="ps")
            for _ in range(N_WARM):
                nc.tensor.matmul(
                    warm_ps[:], zt[:, :128], zt[:], start=True, stop=True
                )

            wt_t = cst.tile([CIN, RBW + NTAP * TAPW], f16, tag="wt")
            rb_t = wt_t[:, 0:RBW].bitcast(f32)  # [128, SPC*E] fp32

            def load_wt_tap(t, eng):
                # first chunk also carries the routing scalars
                lo = 0 if t == 0 else RBW + t * TAPW
                sl = slice(lo, RBW + (t + 1) * TAPW)
                eng.dma_start(wt_t[:, sl], wt_d[:, sl])

            def load_x_chunk(s, xtiles, xch, c, eng):
                r0, nr = xch[c]
                xt = xpool.tile([CIN, nr * WP], f16, tag=f"x{s}_{c}",
                                name=f"x{s}_{c}")
                sl = slice(r0 * WP, (r0 + nr) * WP)
                eng.dma_start(xt[:], xpad_d[s][:, sl])
                xtiles[c] = xt

            # Both rings round-robin at packet granularity and share HBM
            # bandwidth, so each ring's FIFO is ordered by global
            # need-time, with urgent pieces split across the two rings.
            x0t = [None] * len(XCH0)
            x1t = [None] * len(XCH1)
            load_wt_tap(0, nc.scalar)
            load_x_chunk(0, x0t, XCH0, 0, nc.sync)
            load_wt_tap(1, nc.scalar)
            load_x_chunk(0, x0t, XCH0, 1, nc.sync)
            load_wt_tap(2, nc.scalar)
            load_x_chunk(0, x0t, XCH0, 2, nc.sync)
            load_wt_tap(3, nc.scalar)
            load_x_chunk(0, x0t, XCH0, 3, nc.sync)
            load_wt_tap(4, nc.scalar)
            load_wt_tap(5, nc.sync)
            load_wt_tap(6, nc.scalar)
            load_wt_tap(7, nc.sync)
            load_wt_tap(8, nc.scalar)
            load_x_chunk(1, x1t, XCH1, 0, nc.sync)
            load_x_chunk(1, x1t, XCH1, 1, nc.scalar)
            load_x_chunk(1, x1t, XCH1, 2, nc.sync)

            wt3 = wt_t[:, RBW:].rearrange("p (t e c) -> p t e c", t=NTAP, e=E)

            def mix(dst3, s, t0, t1, eng):
                """dst3[:, :, :] = sum_e rb[s,e] * wt[:, t0:t1, e, :]"""
                for e in range(E):
                    sc = rb_t[:, s * E + e : s * E + e + 1]
                    src = wt3[:, t0:t1, e, :]
                    if e == 0:
                        eng.tensor_scalar_mul(dst3, src, sc)
                    else:
                        eng.scalar_tensor_tensor(dst3, src, sc, dst3, MUL, ADD)

            def rhs_ap(xtiles, c, r0, nr, kh, kw):
                xch = XCH0 if xtiles is x0t else XCH1
                loc = r0 - xch[c][0]
                x3 = xtiles[c][:].rearrange("p (h w) -> p h w", w=WP)
                return x3[:, loc + kh : loc + kh + nr, kw : kw + W_SP]

            def store_block(s, ob, ps, r0, nr):
                sl = slice(r0 * W_SP, (r0 + nr) * W_SP)
                nc.vector.tensor_copy(ob[:, sl], ps[:])
                nc.sync.dma_start(out_d[s][:, sl], ob[:, sl])

            # ---- sample 0: tap-outer over 7 live PSUM banks
            wm0 = {}  # tap -> (chunk AP, local tap index)
            ps_map = {}
            for blk in range(NBLK):
                ps_map[blk] = pspool.tile(
                    [COUT, NT], f32, tag="ps", name=f"ps0_{blk}"
                )

            def mix_chunk(c):
                t0, ntc = MIXCH[c]
                wmt = wmp.tile(
                    [CIN, ntc * COUT], f16, tag=f"wmc{c}", name=f"wm0_{c}"
                )
                wm3 = wmt.rearrange("p (t c) -> p t c", t=ntc)
                mix(wm3, 0, t0, t0 + ntc, nc.vector)
                for tt in range(t0, t0 + ntc):
                    wm0[tt] = (wmt, tt - t0)

            def mm0(t, blk):
                kh, kw = divmod(t, KW)
                chunk, loc = wm0[t]
                nc.tensor.matmul(
                    ps_map[blk][:],
                    chunk[:, loc * COUT : (loc + 1) * COUT],
                    rhs_ap(x0t, BLK_CH0[blk], blk * RPB, RPB, kh, kw),
                    start=(t == 0),
                    stop=(t == NTAP - 1),
                    skip_group_check=True,
                )

            # phase 1: taps 0-2 on the first x chunk's blocks — starts as
            # soon as the first tap's weights + first 18 x rows land
            for t in range(3):
                mix_chunk(t)
                for blk in range(2):
                    mm0(t, blk)
            # phase 2: taps 0-2 on the later blocks, block-major to track
            # the arrival of the remaining x chunks
            for blk in range(2, NBLK):
                for t in range(3):
                    mm0(t, blk)
            # phase 3: taps 3-8 everywhere; remaining mix chunks emitted up
            # front so DVE finishes them before they're needed (and before
            # sample 1's mix)
            for c in range(3, len(MIXCH)):
                mix_chunk(c)
            for t in range(3, NTAP):
                for blk in range(NBLK):
                    mm0(t, blk)

            # sample 1 weight mix: runs on DVE during sample 0's stream
            wm1 = wmp.tile([CIN, NTAP * COUT], f16, tag="wm")
            wm1_3 = wm1[:].rearrange("p (t c) -> p t c", t=NTAP)
            mix(wm1_3, 1, 0, NTAP, nc.vector)

            # drain sample 0
            ob0 = opool.tile([COUT, H * W_SP], f32, tag="ob")
            for blk in range(NBLK):
                store_block(0, ob0, ps_map[blk], blk * RPB, RPB)

            # ---- sample 1: block-outer, drains incrementally
            ob1 = opool.tile([COUT, H * W_SP], f32, tag="ob")
            for blk, (r0, nr) in enumerate(BLKS1):
                ps = pspool.tile(
                    [COUT, nr * W_SP], f32, tag="ps", name=f"ps1_{blk}"
                )
                for t in range(NTAP):
                    kh, kw = divmod(t, KW)
                    nc.tensor.matmul(
                        ps[:],
                        wm1[:, t * COUT : (t + 1) * COUT],
                        rhs_ap(x1t, BLK_CH1[blk], r0, nr, kh, kw),
                        start=(t == 0),
                        stop=(t == NTAP - 1),
                    )
                store_block(1, ob1, ps, r0, nr)

    nc.compile()
    return nc


def _get_nc():
    global _cached_nc
    if _cached_nc is None:
        _cached_nc = _build_nc()
    return _cached_nc


def _prep_inputs(x, routing_weights, W):
    x = np.ascontiguousarray(x, dtype=np.float32)
    routing_weights = np.ascontiguousarray(routing_weights, dtype=np.float32)
    W = np.ascontiguousarray(W, dtype=np.float32)

    xpad = np.zeros((B, CIN, HP, WP), np.float16)
    xpad[:, :, 1 : H + 1, 1 : W_SP + 1] = x.reshape(B, CIN, H, W_SP)
    xpad = xpad.reshape(B, CIN, HP * WP)

    # W[e, co, ci, kh, kw] -> wt[ci, (kh, kw, e, co)], with the per-core
    # routing scalars (broadcast over partitions) prepended
    wt = np.ascontiguousarray(
        np.transpose(W, (2, 3, 4, 0, 1)).astype(np.float16)
    ).reshape(CIN, NTAP * E * COUT)

    in_maps = []
    for c in range(NCORES):
        r = routing_weights[c * SPC : (c + 1) * SPC]  # fp32 [SPC, E]
        rb16 = r.reshape(1, SPC * E).view(np.float16)  # fp32 bits as fp16 pairs
        rb = np.broadcast_to(rb16, (128, SPC * E * 2))
        in_maps.append(
            {
                "xpad": xpad[c * SPC : (c + 1) * SPC],
                "wt": np.ascontiguousarray(np.concatenate([rb, wt], axis=1)),
            }
        )
    return in_maps


def _run(in_maps, **kwargs):
    from concourse import bass_utils

    nc = _get_nc()
    res = bass_utils.run_bass_kernel_spmd(
        nc, in_maps, core_ids=list(range(NCORES)), **kwargs
    )
    out = np.concatenate(
        [res.results[c]["out"] for c in range(NCORES)], axis=0
    ).reshape(B, COUT, H, W_SP)
    return out, res


def kernel(x, routing_weights, W):
    in_maps = _prep_inputs(x, routing_weights, W)
    out, _ = _run(in_maps)
    return out


# revision 46
# speedup vs baseline: 1.2850x; 1.0596x over previous
"""CondConv2d (MoE routed conv) Trainium2 kernel.

Math: out[b] = sum_e routing[b,e] * conv3x3(x[b], W[e])
Since the expert mix is linear in W, this equals
    out[b] = conv3x3(x[b], Wmix_b),  Wmix_b = sum_e routing[b,e] * W[e]
which needs 1 conv per sample instead of E=4 (4x less PE work).

Sharding: data-parallel over batch, B=16 -> 2 samples per core on 8 cores.
Weights (all 4 experts, transposed to [ci, tap, e, co] on host) are
replicated; the per-sample mix happens on-device on the Vector engine.

Conv as implicit GEMM: x is zero-padded on host to [ci, 58, 58]; for each
of 9 taps the matmul streams a shifted window of the padded image
(rhs = xpad[:, blk*8+kh : +8, kw : kw+56], N=448) against the tap's mixed
weight slice (lhsT = Wmix[ci, co], K=ci on partitions), accumulating all
9 taps into one PSUM bank (fp32). 7 row-blocks of 8 rows cover the 56
output rows.

Numerics: x and W are rounded to fp16 on the host; matmuls run fp16 at
1 cycle/row with fp32 PSUM accumulation. fp16's 10-bit mantissa gives
~4e-4 L2 relative error on this problem, and halves the load bytes, the
DVE mix cost (16-bit 2x mode), and the weight-load time (FWL) compared
to the fp32 path.

Schedule: sample 0 runs tap-outer (all 7 PSUM banks accumulate one tap at
a time) so matmuls start after only the first tap's weights + first x
rows arrive; loads are chunked and spread across the sync and scalar DMA
rings in global need-time order. Sample 1 runs block-outer (9 taps into
one bank, then drain) so its output streams out incrementally; its weight
mix runs on DVE during sample 0's stream. Dummy matmuls on a zeroed tile
during the load phase keep the PE HAM clock-gate warm (2.4 GHz).
"""

import os
import sys

os.environ.setdefault("MYCRO_LOCAL_CACHE", "1")
for _p in ("/opt/trn_rl_repo",):
    if _p not in sys.path:
        sys.path.insert(0, _p)

import numpy as np

B, CIN, COUT, H, W_SP = 16, 128, 128, 56, 56
E, KH, KW = 4, 3, 3
NCORES = 8
SPC = B // NCORES          # samples per core
HP, WP = H + 2, W_SP + 2   # padded spatial
NTAP = KH * KW
RPB = 8                    # output rows per matmul block
NBLK = H // RPB
NT = RPB * W_SP            # moving-operand free size per matmul (448)
N_WARM = 10                # HAM warm-up dummy matmuls

# sample-0 mix chunks (start_tap, n_taps): per-tap early for latency, then
# wider; each chunk gets its OWN tile (matmul weight reads are tracked
# whole-tile, so chunks sharing a tile serialize behind earlier matmuls)
MIXCH = [(0, 1), (1, 1), (2, 1), (3, 3), (6, 3)]
# fp16 halves the bytes, so transfers are overhead-bound rather than
# bandwidth-bound: load each sample's x as ONE chunk
XCH0 = [(0, 58)]
BLK_CH0 = [0] * NBLK
XCH1 = [(0, 58)]
BLK_CH1 = [0] * NBLK
BLKS1 = [(8 * b, 8) for b in range(NBLK)]
# weight-tap DMA chunks (start_tap, n_taps), need-ordered
WTCH = [(0, 1), (1, 1), (2, 1), (3, 6)]

_cached_nc = None


def _build_nc():
    import concourse.tile as tile
    from concourse import bacc, mybir

    f32 = mybir.dt.float32
    f16 = mybir.dt.float16
    MUL, ADD = mybir.AluOpType.mult, mybir.AluOpType.add

    nc = bacc.Bacc(
        "TRN2", target_bir_lowering=False, debug=False, num_devices=NCORES
    )

    xpad_d = nc.dram_tensor(
        "xpad", [SPC, CIN, HP * WP], f16, kind="ExternalInput"
    ).ap()
    # host layout: [ci, (rb | tap, e, co)] — routing scalars (fp32 bits
    # packed into 2 fp16 slots each; tensor_scalar wants fp32 scalars)
    # share the weight tensor so one DMA delivers both rb and tap 0
    TAPW = E * COUT          # 512 halfs per tap in wt
    RBW = SPC * E * 2        # fp32 scalars as fp16 slot pairs
    wt_d = nc.dram_tensor(
        "wt", [CIN, RBW + NTAP * TAPW], f16, kind="ExternalInput"
    ).ap()
    out_d = nc.dram_tensor(
        "out", [SPC, COUT, H * W_SP], f32, kind="ExternalOutput"
    ).ap()

    with tile.TileContext(nc) as tc:
        with (
            tc.tile_pool(name="const", bufs=1) as cst,
            tc.tile_pool(name="x", bufs=1) as xpool,
            tc.tile_pool(name="wmix", bufs=1) as wmp,
            tc.tile_pool(name="ob", bufs=2) as opool,
            tc.tile_pool(name="ps", bufs=8, space="PSUM") as pspool,
        ):
            # --- HAM warm-up: dummy matmuls on a zeroed tile during loads
            zt = cst.tile([128, 512], f16, tag="zero")
            nc.gpsimd.memset(zt[:], 0.0)
            warm_ps = pspool.tile([128, 512], f32, tag="ps")
            for _ in range(N_WARM):
                nc.tensor.matmul(
                    warm_ps[:], zt[:, :128], zt[:], start=True, stop=True
                )

            wt_t = cst.tile([CIN, RBW + NTAP * TAPW], f16, tag="wt")
            rb_t = wt_t[:, 0:RBW].bitcast(f32)  # [128, SPC*E] fp32

            def load_wt_chunk(t0, ntaps, eng):
                # first chunk also carries the routing scalars
                lo = 0 if t0 == 0 else RBW + t0 * TAPW
                sl = slice(lo, RBW + (t0 + ntaps) * TAPW)
                eng.dma_start(wt_t[:, sl], wt_d[:, sl])

            def load_x_chunk(s, xtiles, xch, c, eng):
                r0, nr = xch[c]
                xt = xpool.tile([CIN, nr * WP], f16, tag=f"x{s}_{c}",
                                name=f"x{s}_{c}")
                sl = slice(r0 * WP, (r0 + nr) * WP)
                eng.dma_start(xt[:], xpad_d[s][:, sl])
                xtiles[c] = xt

            # scalar ring: weights (need-ordered chunks), then sample-1 x.
            # sync ring: sample-0 x, later the output stores.
            x0t = [None] * len(XCH0)
            x1t = [None] * len(XCH1)
            load_x_chunk(0, x0t, XCH0, 0, nc.sync)
            for t0, ntaps in WTCH:
                load_wt_chunk(t0, ntaps, nc.scalar)
            load_x_chunk(1, x1t, XCH1, 0, nc.scalar)

            wt3 = wt_t[:, RBW:].rearrange("p (t e c) -> p t e c", t=NTAP, e=E)

            def mix(dst3, s, t0, t1, eng):
                """dst3[:, :, :] = sum_e rb[s,e] * wt[:, t0:t1, e, :]"""
                for e in range(E):
                    sc = rb_t[:, s * E + e : s * E + e + 1]
                    src = wt3[:, t0:t1, e, :]
                    if e == 0:
                        eng.tensor_scalar_mul(dst3, src, sc)
                    else:
                        eng.scalar_tensor_tensor(dst3, src, sc, dst3, MUL, ADD)

            def rhs_ap(xtiles, c, r0, nr, kh, kw):
                xch = XCH0 if xtiles is x0t else XCH1
                loc = r0 - xch[c][0]
                x3 = xtiles[c][:].rearrange("p (h w) -> p h w", w=WP)
                return x3[:, loc + kh : loc + kh + nr, kw : kw + W_SP]

            def store_block(s, ob, ps, r0, nr):
                sl = slice(r0 * W_SP, (r0 + nr) * W_SP)
                nc.vector.tensor_copy(ob[:, sl], ps[:])
                nc.sync.dma_start(out_d[s][:, sl], ob[:, sl])

            # ---- sample 0: tap-outer over 7 live PSUM banks
            wm0 = {}  # tap -> (chunk AP, local tap index)
            ps_map = {}
            for blk in range(NBLK):
                ps_map[blk] = pspool.tile(
                    [COUT, NT], f32, tag="ps", name=f"ps0_{blk}"
                )

            def mix_chunk(c):
                t0, ntc = MIXCH[c]
                wmt = wmp.tile(
                    [CIN, ntc * COUT], f16, tag=f"wmc{c}", name=f"wm0_{c}"
                )
                wm3 = wmt.rearrange("p (t c) -> p t c", t=ntc)
                mix(wm3, 0, t0, t0 + ntc, nc.vector)
                for tt in range(t0, t0 + ntc):
                    wm0[tt] = (wmt, tt - t0)

            def mm0(t, blk):
                kh, kw = divmod(t, KW)
                chunk, loc = wm0[t]
                nc.tensor.matmul(
                    ps_map[blk][:],
                    chunk[:, loc * COUT : (loc + 1) * COUT],
                    rhs_ap(x0t, BLK_CH0[blk], blk * RPB, RPB, kh, kw),
                    start=(t == 0),
                    stop=(t == NTAP - 1),
                    skip_group_check=True,
                )

            # tap-outer sweep; each tap's mix chunk is emitted before its
            # matmuls (per-tap early for latency, wider later)
            next_chunk = 0
            for t in range(NTAP):
                if next_chunk < len(MIXCH) and MIXCH[next_chunk][0] == t:
                    mix_chunk(next_chunk)
                    next_chunk += 1
                for blk in range(NBLK):
                    mm0(t, blk)

            # sample 1 weight mix: runs on DVE during sample 0's stream
            wm1 = wmp.tile([CIN, NTAP * COUT], f16, tag="wm")
            wm1_3 = wm1[:].rearrange("p (t c) -> p t c", t=NTAP)
            mix(wm1_3, 1, 0, NTAP, nc.vector)

            # drain sample 0
            ob0 = opool.tile([COUT, H * W_SP], f32, tag="ob")
            for blk in range(NBLK):
                store_block(0, ob0, ps_map[blk], blk * RPB, RPB)

            # ---- sample 1: block-outer, drains incrementally
            ob1 = opool.tile([COUT, H * W_SP], f32, tag="ob")
            for blk, (r0, nr) in enumerate(BLKS1):
                ps = pspool.tile(
                    [COUT, nr * W_SP], f32, tag="ps", name=f"ps1_{blk}"
                )
                for t in range(NTAP):
                    kh, kw = divmod(t, KW)
                    nc.tensor.matmul(
                        ps[:],
                        wm1[:, t * COUT : (t + 1) * COUT],
                        rhs_ap(x1t, BLK_CH1[blk], r0, nr, kh, kw),
                        start=(t == 0),
                        stop=(t == NTAP - 1),
                    )
                store_block(1, ob1, ps, r0, nr)

    nc.compile()
    return nc


def _get_nc():
    global _cached_nc
    if _cached_nc is None:
        _cached_nc = _build_nc()
    return _cached_nc


def _prep_inputs(x, routing_weights, W):
    x = np.ascontiguousarray(x, dtype=np.float32)
    routing_weights = np.ascontiguousarray(routing_weights, dtype=np.float32)
    W = np.ascontiguousarray(W, dtype=np.float32)

    xpad = np.zeros((B, CIN, HP, WP), np.float16)
    xpad[:, :, 1 : H + 1, 1 : W_SP + 1] = x.reshape(B, CIN, H, W_SP)
    xpad = xpad.reshape(B, CIN, HP * WP)

    # W[e, co, ci, kh, kw] -> wt[ci, (kh, kw, e, co)], with the per-core
    # routing scalars (broadcast over partitions) prepended
    wt = np.ascontiguousarray(
        np.transpose(W, (2, 3, 4, 0, 1)).astype(np.float16)
    ).reshape(CIN, NTAP * E * COUT)

    in_maps = []
    for c in range(NCORES):
        r = routing_weights[c * SPC : (c + 1) * SPC]  # fp32 [SPC, E]
        rb16 = r.reshape(1, SPC * E).view(np.float16)  # fp32 bits as fp16 pairs
        rb = np.broadcast_to(rb16, (128, SPC * E * 2))
        in_maps.append(
            {
                "xpad": xpad[c * SPC : (c + 1) * SPC],
                "wt": np.ascontiguousarray(np.concatenate([rb, wt], axis=1)),
            }
        )
    return in_maps


def _run(in_maps, **kwargs):
    from concourse import bass_utils

    nc = _get_nc()
    res = bass_utils.run_bass_kernel_spmd(
        nc, in_maps, core_ids=list(range(NCORES)), **kwargs
    )
    out = np.concatenate(
        [res.results[c]["out"] for c in range(NCORES)], axis=0
    ).reshape(B, COUT, H, W_SP)
    return out, res


def kernel(x, routing_weights, W):
    in_maps = _prep_inputs(x, routing_weights, W)
    out, _ = _run(in_maps)
    return out
